# revision 44
# baseline (speedup 1.0000x reference)
"""GNN message-passing kernel for TRN2, one batch element per NeuronCore.

Per-core math (x: [W=2048, C=512], weights replicated, inputs staged bf16):
  cw    = sigmoid(relu(mean_W(x)@avg_w.T) + relu(max_W(x)@max_w.T))   [M=128]
  xxT   = fc_w @ xT + b                         [M, W]
  Q8    = sqrt(cw) * xxT   (fp8e4, zero-padded k-subtile for DoubleRow)
  S     = sigmoid(Q8.T @ Q8) symmetric -> upper trapezoid row blocks;
          deg = rowsums (ACT sigmoid accum_out) + colsums of off-diagonal
          128-blocks (PE ones-matmuls, DVE accumulate).
  d     = deg^-1/2 (DVE Newton);  P = d*xx
  G1    = x @ gcn_w;  T1 = P^T G1;  T2 = -cw*T1
  out_i = G1_i + PT_i.T @ T2  (bf16 store; host casts to f32)

Engine budget: PE is the wall (xx + fp8-DR S + colsums + G1 + T1 + out);
ACT only sigmoids + stats sums; DVE does max-reduce trees, cw/rsqrt chain,
Q8 quantize, deg/d chain, odd-tile adds; Pool does xx bias copies, G1 psum
drains, even-tile copies; DMA (idle mid-kernel) does xx_nat/PT transposes.
"""

from contextlib import ExitStack

import numpy as np

import concourse.bass as bass
import concourse.tile as tile
from concourse import bacc, mybir

f32 = mybir.dt.float32
bf16 = mybir.dt.bfloat16
fp8 = mybir.dt.float8e4
AF = mybir.ActivationFunctionType
ALU = mybir.AluOpType
DR = mybir.MatmulPerfMode.DoubleRow

W, C, M = 2048, 512, 128
CQ = C // 128      # 4 c-chunks
NW = W // 128      # 16 w-chunks
WS = W // 512      # 4 w-slices


def build_nc():
    nc = bacc.Bacc("TRN2", target_bir_lowering=False, debug=False, num_devices=8)

    xT_d = nc.dram_tensor("xT", [C, W], bf16, kind="ExternalInput").ap()
    wcat_d = nc.dram_tensor("wcat", [C, 3 * M], bf16, kind="ExternalInput").ap()
    ident_d = nc.dram_tensor("ident", [128, 128], bf16, kind="ExternalInput").ap()
    fcb_d = nc.dram_tensor("fcb", [M, 1], f32, kind="ExternalInput").ap()
    gcn_d = nc.dram_tensor("gcn", [C, C], bf16, kind="ExternalInput").ap()
    out_d = nc.dram_tensor("out", [W, C], bf16, kind="ExternalOutput").ap()

    with tile.TileContext(nc) as tc, ExitStack() as ctx:
        pool = ctx.enter_context(tc.tile_pool(name="sb", bufs=1))
        sigp = ctx.enter_context(tc.tile_pool(name="sigp", bufs=4))
        outp = ctx.enter_context(tc.tile_pool(name="outp", bufs=5))
        psS = ctx.enter_context(tc.tile_pool(name="psS", bufs=2, space="PSUM"))
        psA = ctx.enter_context(tc.tile_pool(name="psA", bufs=2, space="PSUM"))
        psB = ctx.enter_context(tc.tile_pool(name="psB", bufs=1, space="PSUM"))
        psC = ctx.enter_context(tc.tile_pool(name="psC", bufs=1, space="PSUM"))

        # ---------- persistent SBUF tensors ----------
        identb = pool.tile([128, 128], bf16)
        wcat = pool.tile([128, CQ, 3 * M], bf16)   # [fcwT | avgwT | maxwT]
        fcb = pool.tile([128, 1], f32)
        xT = pool.tile([128, CQ, W], bf16)
        gcn = pool.tile([128, CQ, C], bf16)
        xxT = pool.tile([128, W], bf16)
        xx8 = pool.tile([128, 2, W], fp8)          # k-subtile 1 zeroed (DoubleRow pad)
        dq8 = pool.tile([128, 2, W], fp8)          # cw * xx, fp8 (lhsT side)
        xx_nat = pool.tile([128, NW, 128], bf16)   # w-chunks; scaled by d in place -> P
        G1 = pool.tile([128, NW, C], bf16)
        T2 = pool.tile([128, C], bf16)
        dump = pool.tile([128, W // 2], bf16)      # ACT stats dump target
        mx1 = pool.tile([128, W // 2], bf16)       # max-tree level-1 scratch
        xsum_p = pool.tile([128, CQ, 2], f32)
        xmax_p = pool.tile([128, CQ, 2], f32)
        xmax_f = pool.tile([128, CQ], f32)
        xsum_bf = pool.tile([128, CQ], bf16)
        xmax_bf = pool.tile([128, CQ], bf16)
        a_sb = pool.tile([128, 1], f32)
        m_sb = pool.tile([128, 1], f32)
        cw = pool.tile([128, 1], f32)
        ncw = pool.tile([128, 1], f32)
        ones_bf = pool.tile([128, 1], bf16)
        zeros1 = pool.tile([128, 1], f32)
        scr1 = pool.tile([128, 1], f32)
        deg_parts = pool.tile([128, NW, 2], f32)
        acc_cs = pool.tile([128, NW], f32)
        deg = pool.tile([128, NW], f32)
        y_nr = pool.tile([128, NW], f32)
        y_inv = pool.tile([128, NW], f32)
        G1d = pool.tile([128, NW, C], bf16)
        t_nr = pool.tile([128, NW], f32)
        u_nr = pool.tile([128, NW], f32)

        # Pin the ACT table set (sigmoid_and_others also holds Copy/Relu).
        nc.gpsimd.memset(zeros1[:], 0.0)
        nc.scalar.activation(scr1[:], zeros1[:], AF.Sigmoid)
        nc.vector.memset(y_nr[:], 1.0 / 32.0)
        nc.vector.memset(ones_bf[:], 1.0)
        nc.vector.memset(deg_parts[:].rearrange("p a b -> p (a b)"), 0.0)
        nc.vector.memset(acc_cs[:], 0.0)
        nc.gpsimd.memset(xx8[:, 1, :], 0.0)
        nc.gpsimd.memset(dq8[:, 1, :], 0.0)

        # ---------- loads: ident/fcb/wcat, xT half-chunks (k-major), gcn last ----------
        nc.sync.dma_start(identb[:], ident_d[:])
        nc.sync.dma_start(fcb[:], fcb_d[:])
        for h in range(8):
            k, p = h // 2, h % 2
            nc.sync.dma_start(xT[:, k, bass.ts(p, W // 2)], xT_d[bass.ts(k, 128), bass.ts(p, W // 2)])
        nc.sync.dma_start(wcat[:], wcat_d.rearrange("(k p) m -> p k m", p=128))
        nc.sync.dma_start(gcn[:], gcn_d.rearrange("(k p) c -> p k c", p=128))

        # ---------- PE p-state prewarm (paced by early arrivals) ----------
        pw = psB.tile([128, 512], f32, tag="b")
        for _ in range(6):
            nc.tensor.matmul(pw[:, 0:128], identb[:], identb[:], start=True, stop=True)

        # ---------- stats pipelined with loads ----------
        # sums on ACT (8 ops), maxes on DVE (per-chunk pair-max then reduce)
        for h in range(8):
            k, p = h // 2, h % 2
            sl = xT[:, k, bass.ts(p, W // 2)]
            if k < 3:
                nc.scalar.activation(dump[:], sl, AF.Copy, accum_out=xsum_p[:, k, p : p + 1])
            nc.tensor.matmul(pw[:, 0:128], identb[:], sl[:, 0:128], start=True, stop=True)
            nc.vector.reduce_max(xmax_p[:, k, p : p + 1], sl, axis=mybir.AxisListType.X)
        # chunk 3 sum: Pool pair-adds halves, single ACT accum
        nc.gpsimd.tensor_tensor(mx1[:], xT[:, 3, 0 : W // 2], xT[:, 3, W // 2 : W], op=ALU.add)
        nc.scalar.activation(dump[:], mx1[:], AF.Copy, accum_out=xsum_p[:, 3, 0:1])
        nc.vector.memset(xsum_p[:, 3, 1:2], 0.0)

        # ---------- xxT = fc_w @ xT + b (PE; Pool drains psum with bias) ----------
        for s in range(WS):
            px = psA.tile([128, 512], f32, tag="a")
            for k in range(CQ):
                nc.tensor.matmul(px[:], wcat[:, k, 0:128], xT[:, k, bass.ts(s, 512)], start=(k == 0), stop=(k == CQ - 1))
            nc.scalar.activation(xxT[:, bass.ts(s, 512)], px[:], AF.Identity, bias=fcb[:, 0:1])
            nc.scalar.activation(xx8[:, 0, bass.ts(s, 512)], px[:], AF.Identity, bias=fcb[:, 0:1])

        # ---------- cw chain ----------
        nc.vector.tensor_tensor(xsum_bf[:], xsum_p[:, :, 0:1].rearrange("p a b -> p (a b)"), xsum_p[:, :, 1:2].rearrange("p a b -> p (a b)"), op=ALU.add)
        nc.vector.reduce_max(xmax_bf[:], xmax_p[:], axis=mybir.AxisListType.X)
        pa = psB.tile([128, 512], f32, tag="b")
        for k in range(CQ):
            nc.tensor.matmul(pa[:, 0:1], wcat[:, k, 128:256], xsum_bf[:, k : k + 1], start=(k == 0), stop=(k == CQ - 1))
        pm = psB.tile([128, 512], f32, tag="b")
        for k in range(CQ):
            nc.tensor.matmul(pm[:, 0:1], wcat[:, k, 256:384], xmax_bf[:, k : k + 1], start=(k == 0), stop=(k == CQ - 1))
        nc.vector.tensor_scalar(a_sb[:], pa[:, 0:1], 1.0 / W, 0.0, op0=ALU.mult, op1=ALU.max)
        nc.vector.tensor_scalar(m_sb[:], pm[:, 0:1], 1.0, 0.0, op0=ALU.mult, op1=ALU.max)
        nc.scalar.activation(cw[:], a_sb[:], AF.Sigmoid, bias=m_sb[:, 0:1])
        nc.vector.tensor_scalar_mul(ncw[:], cw[:], -1.0)

        # dq8 = cw * xxT per slice (slice s gates S row-blocks 4s..4s+3)
        for s in range(WS):
            nc.vector.tensor_scalar_mul(dq8[:, 0, bass.ts(s, 512)], xxT[:, bass.ts(s, 512)], cw[:, 0:1])

        pt1 = None

        def xxnat_dma(g):
            for ii in range(4 * g, 4 * g + 4):
                nc.sync.dma_start_transpose(xx_nat[:, ii, :], xxT[:, bass.ts(ii, 128)])

        def d_chain(lo, hi, iters=3):
            """rsqrt(deg) for chunks [lo, hi) on DVE; scale xx_nat -> P in place."""
            sl = slice(lo, hi)
            nc.vector.reduce_sum(deg[:, sl], deg_parts[:, sl, :], axis=mybir.AxisListType.X)
            csl = slice(max(lo, 1), hi)
            nc.vector.tensor_tensor(deg[:, csl], deg[:, csl], acc_cs[:, csl], op=ALU.add)
            for _ in range(iters):
                nc.vector.tensor_tensor(t_nr[:, sl], y_nr[:, sl], y_nr[:, sl], op=ALU.mult)
                nc.vector.scalar_tensor_tensor(u_nr[:, sl], t_nr[:, sl], -0.5, deg[:, sl], op0=ALU.mult, op1=ALU.mult)
                nc.vector.scalar_tensor_tensor(y_nr[:, sl], u_nr[:, sl], 1.5, y_nr[:, sl], op0=ALU.add, op1=ALU.mult)
            nc.vector.tensor_tensor(y_inv[:, sl], deg[:, sl], y_nr[:, sl], op=ALU.mult)
            for i in range(lo, hi):
                nc.vector.tensor_scalar_mul(xx_nat[:, i, :], xx_nat[:, i, :], y_nr[:, i : i + 1])
                nc.vector.tensor_scalar_mul(G1d[:, i, :], G1[:, i, :], y_inv[:, i : i + 1])

        def g1_tile(i):
            pg = psA.tile([128, 512], f32, tag="a")
            for k in range(CQ):
                nc.tensor.matmul(pg[:], xT[:, k, bass.ts(i, 128)], gcn[:, k, :], start=(k == 0), stop=(k == CQ - 1))
            nc.vector.tensor_copy(G1[:, i, :], pg[:])

        def keepalive(lo, hi):
            ka = psA.tile([128, 512], f32, tag="a")
            for i in range(lo, hi):
                nc.tensor.matmul(ka[:, 0:128], identb[:], G1d[:, i, 0:128], start=True, stop=True)

        def t1_mms(lo, hi):
            nonlocal pt1
            if pt1 is None:
                pt1 = psB.tile([128, 512], f32, tag="b")
            for i in range(lo, hi):
                nc.tensor.matmul(pt1[:], xx_nat[:, i, :], G1[:, i, :], start=(i == 0), stop=(i == NW - 1))

        # G1 pacing: 3 tiles per early S iter (gcn lands as the S phase starts)
        g1_sched = {1: [0, 1, 2], 2: [3, 4, 5], 3: [6, 7, 8], 4: [9, 10, 11], 5: [12, 13], 6: [14, 15]}

        # ---------- S phase: fp8-DR upper trapezoid + column sums ----------
        for i in range(NW):
            start_col = 128 * i
            parts = []
            c0 = start_col
            while c0 < W:
                w = min(1024, W - c0)
                parts.append((c0, w))
                c0 += w
            sig_tiles = []
            for pidx, (c0, w) in enumerate(parts):
                ps = psS.tile([128, 1024], f32, tag="s")
                o = 0
                while o < w:
                    n = min(512, w - o)
                    nc.tensor.matmul(
                        ps[:, o : o + n],
                        dq8[:, :, bass.ts(i, 128)],
                        xx8[:, :, c0 + o : c0 + o + n],
                        start=True,
                        stop=True,
                        perf_mode=DR,
                    )
                    o += n
                sg = sigp.tile([128, 1024], bf16, tag="sg")
                nc.scalar.activation(
                    sg[:, 0:w], ps[:, 0:w], AF.Sigmoid, accum_out=deg_parts[:, i, pidx : pidx + 1]
                )
                sig_tiles.append((sg, c0, w))
            if i < NW - 1:
                cs = psC.tile([128, NW], f32, tag="c")
                first = True
                for sg, c0, w in sig_tiles:
                    j0 = max(c0 // 128, i + 1)
                    for j in range(j0, (c0 + w) // 128):
                        nc.tensor.matmul(
                            cs[:, j : j + 1],
                            sg[:, 128 * j - c0 : 128 * (j + 1) - c0],
                            ones_bf[:],
                            start=first,
                            stop=(j == NW - 1),
                        )
                        first = False
                nc.vector.tensor_tensor(
                    acc_cs[:, i + 1 : NW], acc_cs[:, i + 1 : NW], cs[:, i + 1 : NW], op=ALU.add
                )
            for gi in g1_sched.get(i, []):
                g1_tile(gi)
            if i in (0, 1, 3, 4):
                xxnat_dma({0: 0, 1: 1, 3: 2, 4: 3}[i])
            if i in (3, 7, 11):
                g = (i - 3) // 4
                d_chain(4 * g, 4 * g + 4)
            if i in (4, 8, 12):
                g = (i - 4) // 4
                t1_mms(4 * g, 4 * g + 4)
            if i == 13:
                d_chain(12, 14, iters=2)
            if i == 14:
                d_chain(14, 15, iters=2)
                t1_mms(12, 15)

        d_chain(15, 16, iters=2)
        t1_mms(15, 16)
        keepalive(12, 16)

        # T2 = (-cw) * T1
        nc.vector.tensor_scalar_mul(T2[:], pt1[:], ncw[:, 0:1])

        # ---------- out_i = G1_i + PT_i.T @ T2; bf16 store per 256-row pair ----------
        for p in range(8):
            st = outp.tile([128, 2, 512], bf16)
            for q in range(2):
                i = 2 * p + q
                if i % 2 == 0:
                    po = psA.tile([128, 512], f32, tag="a")
                elif i % 4 == 1:
                    po = psB.tile([128, 512], f32, tag="b")
                else:
                    po = psC.tile([128, 512], f32, tag="c")
                nc.tensor.matmul(po[:], xxT[:, bass.ts(i, 128)], T2[:], start=True, stop=False)
                nc.tensor.matmul(po[:], identb[:], G1d[:, i, :], start=False, stop=True)
                # out_i = d * (xx_i @ T2 + G1_i / d)
                if i % 2 == 0:
                    nc.scalar.activation(st[:, q, :], po[:], AF.Identity, scale=y_nr[:, i : i + 1])
                else:
                    nc.vector.tensor_scalar_mul(st[:, q, :], po[:], y_nr[:, i : i + 1])
            nc.sync.dma_start(
                out_d[bass.ts(p, 256), :].rearrange("(q p) c -> p q c", p=128), st[:]
            )

    nc.compile()
    return nc


# ======================================================================
# Harness entry point: full inputs in, full output out.
# ======================================================================

_NC_CACHE = None


def _get_nc():
    global _NC_CACHE
    if _NC_CACHE is None:
        _NC_CACHE = build_nc()
    return _NC_CACHE


def make_in_maps(x, fc_w, fc_b, avg_w, max_w, gcn_w):
    import ml_dtypes

    bf = ml_dtypes.bfloat16
    x = np.asarray(x, dtype=np.float32)
    fc_w = np.asarray(fc_w, dtype=np.float32)
    fc_b = np.asarray(fc_b, dtype=np.float32)
    avg_w = np.asarray(avg_w, dtype=np.float32)
    max_w = np.asarray(max_w, dtype=np.float32)
    gcn_w = np.asarray(gcn_w, dtype=np.float32)
    wcat = np.concatenate([fc_w.T, avg_w.T, max_w.T], axis=1)  # [C, 3M]
    shared = {
        "wcat": np.ascontiguousarray(wcat).astype(bf),
        "fcb": np.ascontiguousarray(fc_b.reshape(M, 1)),
        "ident": np.eye(128, dtype=np.float32).astype(bf),
        "gcn": np.ascontiguousarray(gcn_w).astype(bf),
    }
    return [
        {"xT": np.ascontiguousarray(x[b].T).astype(bf), **shared}
        for b in range(x.shape[0])
    ]


def kernel(x, fc_w, fc_b, avg_w, max_w, gcn_w):
    from concourse.bass_utils import run_bass_kernel_spmd

    nc = _get_nc()
    in_maps = make_in_maps(x, fc_w, fc_b, avg_w, max_w, gcn_w)
    res = run_bass_kernel_spmd(nc, in_maps, list(range(len(in_maps))))
    out = np.stack([res.results[b]["out"] for b in range(len(in_maps))])
    return out.astype(np.float32)


# revision 51
# speedup vs baseline: 1.0034x; 1.0034x over previous
"""GNN message-passing kernel for TRN2, one batch element per NeuronCore.

Per-core math (x: [W=2048, C=512], weights replicated, inputs staged bf16):
  cw    = sigmoid(relu(mean_W(x)@avg_w.T) + relu(max_W(x)@max_w.T))   [M=128]
  xxT   = fc_w @ xT + b                         [M, W]
  Q8    = sqrt(cw) * xxT   (fp8e4, zero-padded k-subtile for DoubleRow)
  S     = sigmoid(Q8.T @ Q8) symmetric -> upper trapezoid row blocks;
          deg = rowsums (ACT sigmoid accum_out) + colsums of off-diagonal
          128-blocks (PE ones-matmuls, DVE accumulate).
  d     = deg^-1/2 (DVE Newton);  P = d*xx
  G1    = x @ gcn_w;  T1 = P^T G1;  T2 = -cw*T1
  out_i = G1_i + PT_i.T @ T2  (bf16 store; host casts to f32)

Engine budget: PE is the wall (xx + fp8-DR S + colsums + G1 + T1 + out);
ACT only sigmoids + stats sums; DVE does max-reduce trees, cw/rsqrt chain,
Q8 quantize, deg/d chain, odd-tile adds; Pool does xx bias copies, G1 psum
drains, even-tile copies; DMA (idle mid-kernel) does xx_nat/PT transposes.
"""

from contextlib import ExitStack

import numpy as np

import concourse.bass as bass
import concourse.tile as tile
from concourse import bacc, mybir

f32 = mybir.dt.float32
bf16 = mybir.dt.bfloat16
fp8 = mybir.dt.float8e4
AF = mybir.ActivationFunctionType
ALU = mybir.AluOpType
DR = mybir.MatmulPerfMode.DoubleRow

W, C, M = 2048, 512, 128
CQ = C // 128      # 4 c-chunks
NW = W // 128      # 16 w-chunks
WS = W // 512      # 4 w-slices


def build_nc():
    nc = bacc.Bacc("TRN2", target_bir_lowering=False, debug=False, num_devices=8)

    xT_d = nc.dram_tensor("xT", [C, W], bf16, kind="ExternalInput").ap()
    wcat_d = nc.dram_tensor("wcat", [C, 3 * M], bf16, kind="ExternalInput").ap()
    ident_d = nc.dram_tensor("ident", [128, 128], bf16, kind="ExternalInput").ap()
    fcb_d = nc.dram_tensor("fcb", [M, 1], f32, kind="ExternalInput").ap()
    gcn_d = nc.dram_tensor("gcn", [C, C], bf16, kind="ExternalInput").ap()
    out_d = nc.dram_tensor("out", [W, C], bf16, kind="ExternalOutput").ap()

    with tile.TileContext(nc) as tc, ExitStack() as ctx:
        pool = ctx.enter_context(tc.tile_pool(name="sb", bufs=1))
        sigp = ctx.enter_context(tc.tile_pool(name="sigp", bufs=4))
        outp = ctx.enter_context(tc.tile_pool(name="outp", bufs=5))
        psS = ctx.enter_context(tc.tile_pool(name="psS", bufs=2, space="PSUM"))
        psA = ctx.enter_context(tc.tile_pool(name="psA", bufs=2, space="PSUM"))
        psB = ctx.enter_context(tc.tile_pool(name="psB", bufs=1, space="PSUM"))
        psC = ctx.enter_context(tc.tile_pool(name="psC", bufs=1, space="PSUM"))

        # ---------- persistent SBUF tensors ----------
        identb = pool.tile([128, 128], bf16)
        wcat = pool.tile([128, CQ, 3 * M], bf16)   # [fcwT | avgwT | maxwT]
        fcb = pool.tile([128, 1], f32)
        xT = pool.tile([128, CQ, W], bf16)
        gcn = pool.tile([128, CQ, C], bf16)
        xxT = pool.tile([128, W], bf16)
        xx8 = pool.tile([128, 2, W], fp8)          # k-subtile 1 zeroed (DoubleRow pad)
        dq8 = pool.tile([128, 2, W], fp8)          # cw * xx, fp8 (lhsT side)
        xx_nat = pool.tile([128, NW, 128], bf16)   # w-chunks; scaled by d in place -> P
        G1 = pool.tile([128, NW, C], bf16)
        T2 = pool.tile([128, C], bf16)
        dump = pool.tile([128, W // 2], bf16)      # ACT stats dump target
        mx1 = pool.tile([128, W // 2], bf16)       # max-tree level-1 scratch
        xsum_p = pool.tile([128, CQ, 2], f32)
        xmax_p = pool.tile([128, CQ, 2], f32)
        xmax_f = pool.tile([128, CQ], f32)
        xsum_bf = pool.tile([128, CQ], bf16)
        xmax_bf = pool.tile([128, CQ], bf16)
        a_sb = pool.tile([128, 1], f32)
        m_sb = pool.tile([128, 1], f32)
        cw = pool.tile([128, 1], f32)
        ncw = pool.tile([128, 1], f32)
        ones_bf = pool.tile([128, 1], bf16)
        zeros1 = pool.tile([128, 1], f32)
        scr1 = pool.tile([128, 1], f32)
        deg_parts = pool.tile([128, NW, 2], f32)
        acc_cs = pool.tile([128, NW], f32)
        deg = pool.tile([128, NW], f32)
        y_nr = pool.tile([128, NW], f32)
        y_inv = pool.tile([128, NW], f32)
        G1d = pool.tile([128, NW, C], bf16)
        t_nr = pool.tile([128, NW], f32)
        u_nr = pool.tile([128, NW], f32)

        # Pin the ACT table set (sigmoid_and_others also holds Copy/Relu).
        nc.gpsimd.memset(zeros1[:], 0.0)
        nc.scalar.activation(scr1[:], zeros1[:], AF.Sigmoid)
        nc.vector.memset(y_nr[:], 1.0 / 32.0)
        nc.vector.memset(ones_bf[:], 1.0)
        nc.vector.memset(deg_parts[:].rearrange("p a b -> p (a b)"), 0.0)
        nc.vector.memset(acc_cs[:], 0.0)
        nc.gpsimd.memset(xx8[:, 1, :], 0.0)
        nc.gpsimd.memset(dq8[:, 1, :], 0.0)

        # ---------- loads: ident/fcb/wcat, xT half-chunks (k-major), gcn last ----------
        nc.sync.dma_start(identb[:], ident_d[:])
        nc.sync.dma_start(fcb[:], fcb_d[:])
        for h in range(8):
            k, p = h // 2, h % 2
            nc.sync.dma_start(xT[:, k, bass.ts(p, W // 2)], xT_d[bass.ts(k, 128), bass.ts(p, W // 2)])
        nc.sync.dma_start(wcat[:], wcat_d.rearrange("(k p) m -> p k m", p=128))
        nc.sync.dma_start(gcn[:], gcn_d.rearrange("(k p) c -> p k c", p=128))

        # ---------- PE p-state prewarm (paced by early arrivals) ----------
        pw = psB.tile([128, 512], f32, tag="b")
        for _ in range(6):
            nc.tensor.matmul(pw[:, 0:128], identb[:], identb[:], start=True, stop=True)

        # ---------- stats pipelined with loads ----------
        # sums on ACT (8 ops), maxes on DVE (per-chunk pair-max then reduce)
        for h in range(8):
            k, p = h // 2, h % 2
            sl = xT[:, k, bass.ts(p, W // 2)]
            if k < 3:
                nc.scalar.activation(dump[:], sl, AF.Copy, accum_out=xsum_p[:, k, p : p + 1])
            nc.tensor.matmul(pw[:, 0:128], identb[:], sl[:, 0:128], start=True, stop=True)
            nc.vector.reduce_max(xmax_p[:, k, p : p + 1], sl, axis=mybir.AxisListType.X)
        # chunk 3 sum: Pool pair-adds halves, single ACT accum
        nc.gpsimd.tensor_tensor(mx1[:], xT[:, 3, 0 : W // 2], xT[:, 3, W // 2 : W], op=ALU.add)
        nc.scalar.activation(dump[:], mx1[:], AF.Copy, accum_out=xsum_p[:, 3, 0:1])
        nc.vector.memset(xsum_p[:, 3, 1:2], 0.0)

        # ---------- xxT = fc_w @ xT + b (PE; Pool drains psum with bias) ----------
        for s in range(WS):
            px = psA.tile([128, 512], f32, tag="a")
            for k in range(CQ):
                nc.tensor.matmul(px[:], wcat[:, k, 0:128], xT[:, k, bass.ts(s, 512)], start=(k == 0), stop=(k == CQ - 1))
            nc.scalar.activation(xxT[:, bass.ts(s, 512)], px[:], AF.Identity, bias=fcb[:, 0:1])
            if s < 1:
                nc.scalar.activation(xx8[:, 0, bass.ts(s, 512)], px[:], AF.Identity, bias=fcb[:, 0:1])
            else:
                nc.vector.tensor_scalar_add(xx8[:, 0, bass.ts(s, 512)], px[:], fcb[:, 0:1])

        # ---------- cw chain ----------
        nc.vector.tensor_tensor(xsum_bf[:], xsum_p[:, :, 0:1].rearrange("p a b -> p (a b)"), xsum_p[:, :, 1:2].rearrange("p a b -> p (a b)"), op=ALU.add)
        nc.vector.reduce_max(xmax_bf[:], xmax_p[:], axis=mybir.AxisListType.X)
        pa = psB.tile([128, 512], f32, tag="b")
        for k in range(CQ):
            nc.tensor.matmul(pa[:, 0:1], wcat[:, k, 128:256], xsum_bf[:, k : k + 1], start=(k == 0), stop=(k == CQ - 1))
        pm = psB.tile([128, 512], f32, tag="b")
        for k in range(CQ):
            nc.tensor.matmul(pm[:, 0:1], wcat[:, k, 256:384], xmax_bf[:, k : k + 1], start=(k == 0), stop=(k == CQ - 1))
        nc.vector.tensor_scalar(a_sb[:], pa[:, 0:1], 1.0 / W, 0.0, op0=ALU.mult, op1=ALU.max)
        nc.vector.tensor_scalar(m_sb[:], pm[:, 0:1], 1.0, 0.0, op0=ALU.mult, op1=ALU.max)
        nc.scalar.activation(cw[:], a_sb[:], AF.Sigmoid, bias=m_sb[:, 0:1])
        nc.vector.tensor_scalar_mul(ncw[:], cw[:], -1.0)

        # dq8 = cw * xxT per slice (slice s gates S row-blocks 4s..4s+3)
        for s in range(WS):
            nc.vector.tensor_scalar_mul(dq8[:, 0, bass.ts(s, 512)], xxT[:, bass.ts(s, 512)], cw[:, 0:1])

        pt1 = None

        def xxnat_dma(g):
            for ii in range(4 * g, 4 * g + 4):
                nc.sync.dma_start_transpose(xx_nat[:, ii, :], xxT[:, bass.ts(ii, 128)])

        def d_chain(lo, hi, iters=3):
            """rsqrt(deg) for chunks [lo, hi) on DVE; scale xx_nat -> P in place."""
            sl = slice(lo, hi)
            nc.vector.reduce_sum(deg[:, sl], deg_parts[:, sl, :], axis=mybir.AxisListType.X)
            csl = slice(max(lo, 1), hi)
            nc.vector.tensor_tensor(deg[:, csl], deg[:, csl], acc_cs[:, csl], op=ALU.add)
            for _ in range(iters):
                nc.vector.tensor_tensor(t_nr[:, sl], y_nr[:, sl], y_nr[:, sl], op=ALU.mult)
                nc.vector.scalar_tensor_tensor(u_nr[:, sl], t_nr[:, sl], -0.5, deg[:, sl], op0=ALU.mult, op1=ALU.mult)
                nc.vector.scalar_tensor_tensor(y_nr[:, sl], u_nr[:, sl], 1.5, y_nr[:, sl], op0=ALU.add, op1=ALU.mult)
            nc.vector.tensor_tensor(y_inv[:, sl], deg[:, sl], y_nr[:, sl], op=ALU.mult)
            for i in range(lo, hi):
                nc.vector.tensor_scalar_mul(xx_nat[:, i, :], xx_nat[:, i, :], y_nr[:, i : i + 1])
                nc.vector.tensor_scalar_mul(G1d[:, i, :], G1[:, i, :], y_inv[:, i : i + 1])

        def g1_tile(i):
            pg = psA.tile([128, 512], f32, tag="a")
            for k in range(CQ):
                nc.tensor.matmul(pg[:], xT[:, k, bass.ts(i, 128)], gcn[:, k, :], start=(k == 0), stop=(k == CQ - 1))
            nc.vector.tensor_copy(G1[:, i, :], pg[:])

        def keepalive(lo, hi):
            ka = psA.tile([128, 512], f32, tag="a")
            for i in range(lo, hi):
                nc.tensor.matmul(ka[:, 0:128], identb[:], G1d[:, i, 0:128], start=True, stop=True)

        def t1_mms(lo, hi):
            nonlocal pt1
            if pt1 is None:
                pt1 = psB.tile([128, 512], f32, tag="b")
            for i in range(lo, hi):
                nc.tensor.matmul(pt1[:], xx_nat[:, i, :], G1[:, i, :], start=(i == 0), stop=(i == NW - 1))

        # G1 pacing: 3 tiles per early S iter (gcn lands as the S phase starts)
        g1_sched = {1: [0, 1, 2], 2: [3, 4, 5], 3: [6, 7, 8], 4: [9, 10, 11], 5: [12, 13], 6: [14, 15]}

        # ---------- S phase: fp8-DR upper trapezoid + column sums ----------
        for i in range(NW):
            start_col = 128 * i
            parts = []
            c0 = start_col
            while c0 < W:
                w = min(1024, W - c0)
                parts.append((c0, w))
                c0 += w
            sig_tiles = []
            for pidx, (c0, w) in enumerate(parts):
                ps = psS.tile([128, 1024], f32, tag="s")
                o = 0
                while o < w:
                    n = min(512, w - o)
                    nc.tensor.matmul(
                        ps[:, o : o + n],
                        dq8[:, :, bass.ts(i, 128)],
                        xx8[:, :, c0 + o : c0 + o + n],
                        start=True,
                        stop=True,
                        perf_mode=DR,
                    )
                    o += n
                sg = sigp.tile([128, 1024], bf16, tag="sg")
                nc.scalar.activation(
                    sg[:, 0:w], ps[:, 0:w], AF.Sigmoid, accum_out=deg_parts[:, i, pidx : pidx + 1]
                )
                sig_tiles.append((sg, c0, w))
            if i < NW - 1:
                cs = psC.tile([128, NW], f32, tag="c")
                first = True
                for sg, c0, w in sig_tiles:
                    j0 = max(c0 // 128, i + 1)
                    for j in range(j0, (c0 + w) // 128):
                        nc.tensor.matmul(
                            cs[:, j : j + 1],
                            sg[:, 128 * j - c0 : 128 * (j + 1) - c0],
                            ones_bf[:],
                            start=first,
                            stop=(j == NW - 1),
                        )
                        first = False
                nc.vector.tensor_tensor(
                    acc_cs[:, i + 1 : NW], acc_cs[:, i + 1 : NW], cs[:, i + 1 : NW], op=ALU.add
                )
            for gi in g1_sched.get(i, []):
                g1_tile(gi)
            if i in (0, 1, 3, 4):
                xxnat_dma({0: 0, 1: 1, 3: 2, 4: 3}[i])
            if i in (3, 7, 11):
                g = (i - 3) // 4
                d_chain(4 * g, 4 * g + 4)
            if i in (4, 8, 12):
                g = (i - 4) // 4
                t1_mms(4 * g, 4 * g + 4)
            if i == 13:
                d_chain(12, 14, iters=2)
            if i == 14:
                d_chain(14, 15, iters=2)
                t1_mms(12, 15)

        d_chain(15, 16, iters=2)
        t1_mms(15, 16)
        keepalive(12, 16)

        # T2 = (-cw) * T1
        nc.vector.tensor_scalar_mul(T2[:], pt1[:], ncw[:, 0:1])

        # ---------- out_i = G1_i + PT_i.T @ T2; bf16 store per 256-row pair ----------
        for p in range(8):
            st = outp.tile([128, 2, 512], bf16)
            for q in range(2):
                i = 2 * p + q
                if i % 2 == 0:
                    po = psA.tile([128, 512], f32, tag="a")
                elif i % 4 == 1:
                    po = psB.tile([128, 512], f32, tag="b")
                else:
                    po = psC.tile([128, 512], f32, tag="c")
                nc.tensor.matmul(po[:], xxT[:, bass.ts(i, 128)], T2[:], start=True, stop=False)
                nc.tensor.matmul(po[:], identb[:], G1d[:, i, :], start=False, stop=True)
                # out_i = d * (xx_i @ T2 + G1_i / d)
                if i % 2 == 0:
                    nc.scalar.activation(st[:, q, :], po[:], AF.Identity, scale=y_nr[:, i : i + 1])
                else:
                    nc.vector.tensor_scalar_mul(st[:, q, :], po[:], y_nr[:, i : i + 1])
            nc.sync.dma_start(
                out_d[bass.ts(p, 256), :].rearrange("(q p) c -> p q c", p=128), st[:]
            )

    nc.compile()
    return nc


# ======================================================================
# Harness entry point: full inputs in, full output out.
# ======================================================================

_NC_CACHE = None


def _get_nc():
    global _NC_CACHE
    if _NC_CACHE is None:
        _NC_CACHE = build_nc()
    return _NC_CACHE


def make_in_maps(x, fc_w, fc_b, avg_w, max_w, gcn_w):
    import ml_dtypes

    bf = ml_dtypes.bfloat16
    x = np.asarray(x, dtype=np.float32)
    fc_w = np.asarray(fc_w, dtype=np.float32)
    fc_b = np.asarray(fc_b, dtype=np.float32)
    avg_w = np.asarray(avg_w, dtype=np.float32)
    max_w = np.asarray(max_w, dtype=np.float32)
    gcn_w = np.asarray(gcn_w, dtype=np.float32)
    wcat = np.concatenate([fc_w.T, avg_w.T, max_w.T], axis=1)  # [C, 3M]
    shared = {
        "wcat": np.ascontiguousarray(wcat).astype(bf),
        "fcb": np.ascontiguousarray(fc_b.reshape(M, 1)),
        "ident": np.eye(128, dtype=np.float32).astype(bf),
        "gcn": np.ascontiguousarray(gcn_w).astype(bf),
    }
    return [
        {"xT": np.ascontiguousarray(x[b].T).astype(bf), **shared}
        for b in range(x.shape[0])
    ]


def kernel(x, fc_w, fc_b, avg_w, max_w, gcn_w):
    from concourse.bass_utils import run_bass_kernel_spmd

    nc = _get_nc()
    in_maps = make_in_maps(x, fc_w, fc_b, avg_w, max_w, gcn_w)
    res = run_bass_kernel_spmd(nc, in_maps, list(range(len(in_maps))))
    out = np.stack([res.results[b]["out"] for b in range(len(in_maps))])
    return out.astype(np.float32)
